# revision 4
# baseline (speedup 1.0000x reference)
"""Trainium2 Bass kernel for CustomMHA (B=4, T=2048, D=1024, H=16) on 8 NeuronCores.

Sharding: core c handles batch b=c//2, head-group hg=c%2 (8 heads, 512 cols).
Per core:
  - projections: qT/kT [512j, 2048t] (transposed), v' [2048s', 520] (natural,
    with a ones-column per head for the softmax denominator), all f32r matmuls.
  - per (head, t-chunk): S.T = k·q scores transposed [s',t] -> exp (ACT) ->
    z' = v'.T @ E accumulated in PSUM ([65, t]; row 64 = denominator),
    f[t] = gate/denom broadcast to [128,t], A.T tile = E*f -> DMA out,
    z = z'[0:64]*f -> DRAM scratch.
  - out = z @ Wo_hg.T partial, summed on host (row-parallel Wo).
A is returned as a zero-copy transposed view of the device's [s',t] layout.
"""

import os
import numpy as np

B, T, D, H = 4, 2048, 1024, 16
Dh = D // H            # 64
NCORES = 8
HPC = H // 2           # heads per core = 8
DHG = D // 2           # head-group width = 512
TC = 1024              # t-chunk
NTC = T // TC          # 2 chunks
NSJ = T // 128         # 16 s'-tiles
NJT = DHG // 128       # 4 j-tiles

_CACHE = {}
LAST_RESULTS = None


def _maybe_install_profhook():
    """Best-effort NTFF profile hook (image lacks antenv.axon_hooks)."""
    import sys
    import types
    try:
        import antenv
        if "antenv.axon_hooks" in sys.modules:
            return
        from trn_agent_boot.trn_boot import _ntff_profile_via_ctypes
        hook = _ntff_profile_via_ctypes("/opt/axon/libaxon_pjrt.so")
        mod = types.ModuleType("antenv.axon_hooks")
        mod.get_axon_ntff_profile_hook = lambda: hook
        mod.set_axon_ntff_profile_hook = lambda h: None
        sys.modules["antenv.axon_hooks"] = mod
        antenv.axon_hooks = mod
    except Exception:
        pass


def _build():
    from contextlib import ExitStack
    from concourse import bacc
    import concourse.tile as tile
    from concourse import mybir

    f32 = mybir.dt.float32
    f32r = mybir.dt.float32r
    Exp = mybir.ActivationFunctionType.Exp
    Ident = mybir.ActivationFunctionType.Identity
    MUL = mybir.AluOpType.mult

    nc = bacc.Bacc("TRN2", target_bir_lowering=False, debug=False)

    xT = nc.dram_tensor("xT", [D, T], f32r, kind="ExternalInput").ap()
    wqT = nc.dram_tensor("wqT", [D, DHG], f32r, kind="ExternalInput").ap()
    wkT = nc.dram_tensor("wkT", [D, DHG], f32r, kind="ExternalInput").ap()
    wvT = nc.dram_tensor("wvT", [D, DHG], f32r, kind="ExternalInput").ap()
    woT = nc.dram_tensor("woT", [DHG, D], f32r, kind="ExternalInput").ap()
    bq_t = nc.dram_tensor("bq_t", [128, NJT], f32, kind="ExternalInput").ap()
    bk_t = nc.dram_tensor("bk_t", [128, NJT], f32, kind="ExternalInput").ap()
    gates_t = nc.dram_tensor("gates_t", [1, HPC], f32, kind="ExternalInput").ap()

    at_out = nc.dram_tensor("at_out", [HPC, T, T], f32, kind="ExternalOutput").ap()
    o_out = nc.dram_tensor("o_out", [T, D], f32, kind="ExternalOutput").ap()
    z_scr = nc.dram_tensor("z_scr", [DHG, T], f32r).ap()

    with tile.TileContext(nc) as tc, ExitStack() as ctx:
        # ---- persistent pools (projections output, consumed by attention) ----
        per = ctx.enter_context(tc.tile_pool(name="per", bufs=1))
        qt_p = ctx.enter_context(tc.tile_pool(name="qt", bufs=NJT))
        kt_p = ctx.enter_context(tc.tile_pool(name="kt", bufs=NJT))
        vp_p = ctx.enter_context(tc.tile_pool(name="vp", bufs=NSJ))

        gt = per.tile([1, HPC], f32, tag="gt")
        nc.sync.dma_start(gt[:], gates_t[:])
        bqt = per.tile([128, NJT], f32, tag="bqt")
        nc.sync.dma_start(bqt[:], bq_t[:])
        bkt = per.tile([128, NJT], f32, tag="bkt")
        nc.sync.dma_start(bkt[:], bk_t[:])

        qt = [qt_p.tile([128, T], f32r, tag="qt", name=f"qt{i}") for i in range(NJT)]
        kt = [kt_p.tile([128, T], f32r, tag="kt", name=f"kt{i}") for i in range(NJT)]
        vp = [vp_p.tile([128, HPC * 65], f32r, tag="vp", name=f"vp{i}") for i in range(NSJ)]

        # ---- phase 1: projections ----
        with tc.tile_pool(name="xt", bufs=8) as xt_p, \
             tc.tile_pool(name="w", bufs=16) as w_p, \
             tc.tile_pool(name="pp", bufs=4, space="PSUM") as pp:
            xt = []
            for kc in range(8):
                t_ = xt_p.tile([128, T], f32r, tag="xt", name=f"xt{kc}")
                nc.sync.dma_start(t_[:], xT[kc * 128:(kc + 1) * 128, :])
                xt.append(t_)

            def load_w(w_dram):
                ws = []
                for kc in range(8):
                    t_ = w_p.tile([128, DHG], f32r, tag="w", name=f"w{kc}")
                    nc.sync.dma_start(t_[:], w_dram[kc * 128:(kc + 1) * 128, :])
                    ws.append(t_)
                return ws

            # qT[j, t] / kT[j, t] projections (out transposed)
            for W, bias, dst in ((load_w(wqT), bqt, qt), (load_w(wkT), bkt, kt)):
                for jt in range(NJT):
                    for tcc in range(4):
                        ps = pp.tile([128, 512], f32, tag="pp")
                        for kc in range(8):
                            nc.tensor.matmul(
                                ps[:],
                                W[kc][:, jt * 128:(jt + 1) * 128],
                                xt[kc][:, tcc * 512:(tcc + 1) * 512],
                                start=(kc == 0), stop=(kc == 7),
                            )
                        nc.scalar.activation(
                            dst[jt][:, tcc * 512:(tcc + 1) * 512],
                            ps[:], Ident, bias=bias[:, jt:jt + 1], scale=1.0,
                        )
            # v natural [t, j], into per-head 65-stride layout + ones columns
            wv = load_w(wvT)
            for tt in range(NSJ):
                ps = pp.tile([128, 512], f32, tag="pp")
                for kc in range(8):
                    nc.tensor.matmul(
                        ps[:],
                        xt[kc][:, tt * 128:(tt + 1) * 128],
                        wv[kc][:],
                        start=(kc == 0), stop=(kc == 7),
                    )
                vv = vp[tt][:].rearrange("p (h c) -> p h c", c=65)[:, :, 0:64]
                nc.scalar.copy(vv, ps[:].rearrange("p (h c) -> p h c", c=64))
                for h in range(HPC):
                    nc.vector.memset(vp[tt][:, h * 65 + 64: h * 65 + 65].bitcast(f32), 1.0)

        # ---- phase 2: attention ----
        with tc.tile_pool(name="st", bufs=2, space="PSUM") as st_p, \
             tc.tile_pool(name="zt", bufs=2, space="PSUM") as zt_p, \
             tc.tile_pool(name="e", bufs=17) as e_p, \
             tc.tile_pool(name="fb", bufs=2) as fb_p, \
             tc.tile_pool(name="ast", bufs=3) as ast_p, \
             tc.tile_pool(name="zs", bufs=2) as zs_p, \
             tc.tile_pool(name="fr", bufs=2) as fr_p:
            zstage = {}
            for h in range(HPC):
                jt, po = h // 2, (h % 2) * 64
                for c in range(NTC):
                    if h % 2 == 0:
                        zstage[(jt, c)] = zs_p.tile([128, TC], f32r, tag="zs", name=f"zs{jt}_{c}")
                    zt_ps = zt_p.tile([65, TC], f32, tag="zt")
                    e_tiles = []
                    for sj in range(NSJ):
                        st_ps = st_p.tile([128, TC], f32, tag="st")
                        for nn in range(2):
                            nc.tensor.matmul(
                                st_ps[:, nn * 512:(nn + 1) * 512],
                                kt[jt][po:po + 64, sj * 128:(sj + 1) * 128],
                                qt[jt][po:po + 64, c * TC + nn * 512: c * TC + (nn + 1) * 512],
                                start=True, stop=True,
                            )
                        e_t = e_p.tile([128, TC], f32r, tag="e")
                        nc.scalar.activation(e_t[:], st_ps[:], Exp, scale=0.125)
                        for nn in range(2):
                            nc.tensor.matmul(
                                zt_ps[:, nn * 512:(nn + 1) * 512],
                                vp[sj][:, h * 65:(h + 1) * 65],
                                e_t[:, nn * 512:(nn + 1) * 512],
                                start=(sj == 0), stop=(sj == NSJ - 1),
                            )
                        e_tiles.append(e_t)
                    recip = fr_p.tile([1, TC], f32, tag="recip")
                    nc.vector.reciprocal(recip[:], zt_ps[64:65, :])
                    nc.vector.tensor_scalar(recip[:], recip[:], gt[0:1, h:h + 1], None, MUL)
                    fb = fb_p.tile([128, TC], f32, tag="fb")
                    nc.gpsimd.partition_broadcast(fb[:], recip[:])
                    for sj in range(NSJ):
                        a_t = ast_p.tile([128, TC], f32, tag="ast")
                        eng = nc.vector if sj % 2 == 0 else nc.gpsimd
                        eng.tensor_tensor(
                            out=a_t[:], in0=e_tiles[sj][:].bitcast(f32), in1=fb[:], op=MUL
                        )
                        nc.sync.dma_start(
                            at_out[h, sj * 128:(sj + 1) * 128, c * TC:(c + 1) * TC], a_t[:]
                        )
                    nc.vector.tensor_tensor(
                        out=zstage[(jt, c)][po:po + 64, :].bitcast(f32),
                        in0=zt_ps[0:64, :], in1=fb[0:64, :], op=MUL,
                    )
                    if h % 2 == 1:
                        nc.sync.dma_start(
                            z_scr[jt * 128:(jt + 1) * 128, c * TC:(c + 1) * TC],
                            zstage[(jt, c)][:],
                        )

        # ---- phase 3: output projection (partial; host sums the two cores) ----
        with tc.tile_pool(name="zi", bufs=4) as zi_p, \
             tc.tile_pool(name="wo", bufs=4) as wo_p, \
             tc.tile_pool(name="ost", bufs=3) as ost_p, \
             tc.tile_pool(name="op", bufs=4, space="PSUM") as op_p:
            zi, wo = [], []
            for jt in range(NJT):
                t_ = zi_p.tile([128, T], f32r, tag="zi", name=f"zi{jt}")
                nc.sync.dma_start(t_[:], z_scr[jt * 128:(jt + 1) * 128, :])
                zi.append(t_)
                t2 = wo_p.tile([128, D], f32r, tag="wo", name=f"wo{jt}")
                nc.sync.dma_start(t2[:], woT[jt * 128:(jt + 1) * 128, :])
                wo.append(t2)
            for tt in range(NSJ):
                ost = ost_p.tile([128, D], f32, tag="ost")
                for dc in range(2):
                    ps = op_p.tile([128, 512], f32, tag="op")
                    for jt in range(NJT):
                        nc.tensor.matmul(
                            ps[:],
                            zi[jt][:, tt * 128:(tt + 1) * 128],
                            wo[jt][:, dc * 512:(dc + 1) * 512],
                            start=(jt == 0), stop=(jt == NJT - 1),
                        )
                    nc.scalar.copy(ost[:, dc * 512:(dc + 1) * 512], ps[:])
                nc.sync.dma_start(o_out[tt * 128:(tt + 1) * 128, :], ost[:])

    nc.compile()
    return nc


def _get_compiled():
    if "nc" not in _CACHE:
        _CACHE["nc"] = _build()
    return _CACHE["nc"]


def kernel(x, gates, Wq, bq, Wk, bk, Wv, bv, Wo, bo):
    global LAST_RESULTS
    if os.environ.get("BASS_TRACE"):
        _maybe_install_profhook()
    from concourse.bass_utils import run_bass_kernel_spmd

    nc = _get_compiled()
    x = np.asarray(x, np.float32)
    gates_v = np.asarray(gates, np.float32)
    Wq_v, Wk_v, Wv_v, Wo_v = (np.asarray(w, np.float32) for w in (Wq, Wk, Wv, Wo))
    bq_v, bk_v, bv_v, bo_v = (np.asarray(b_, np.float32) for b_ in (bq, bk, bv, bo))

    xTs = [np.ascontiguousarray(x[b].T) for b in range(B)]
    in_maps = []
    for c in range(NCORES):
        b, hg = c // 2, c % 2
        sl = slice(hg * DHG, (hg + 1) * DHG)
        in_maps.append({
            "xT": xTs[b],
            "wqT": np.ascontiguousarray(Wq_v[sl, :].T),
            "wkT": np.ascontiguousarray(Wk_v[sl, :].T),
            "wvT": np.ascontiguousarray(Wv_v[sl, :].T),
            "woT": np.ascontiguousarray(Wo_v[:, sl].T),
            "bq_t": np.ascontiguousarray(bq_v[sl].reshape(NJT, 128).T),
            "bk_t": np.ascontiguousarray(bk_v[sl].reshape(NJT, 128).T),
            "gates_t": np.ascontiguousarray(gates_v[b, hg * HPC:(hg + 1) * HPC].reshape(1, HPC)),
        })

    res = run_bass_kernel_spmd(nc, in_maps, core_ids=list(range(NCORES)))
    LAST_RESULTS = res

    out = np.empty((B, T, D), np.float32)
    AT = np.empty((B, H, T, T), np.float32)
    for b in range(B):
        r0, r1 = res.results[2 * b], res.results[2 * b + 1]
        out[b] = r0["o_out"] + r1["o_out"] + bo_v
        # exact bv correction: z += softmax-row-sum (=1) * gate * bv
        out[b] += Wo_v @ (np.repeat(gates_v[b], Dh) * bv_v)
        AT[b, 0:HPC] = r0["at_out"]
        AT[b, HPC:H] = r1["at_out"]
    return out, AT.transpose(0, 1, 3, 2)


# revision 6
# speedup vs baseline: 1.0343x; 1.0343x over previous
"""Trainium2 Bass kernel for CustomMHA (B=4, T=2048, D=1024, H=16) on 8 NeuronCores.

Sharding: core c handles batch b=c//2, head-group hg=c%2 (8 heads, 512 cols).
Per core:
  - projections: qT/kT [512j, 2048t] (transposed), v' [2048s', 520] (natural,
    with a ones-column per head for the softmax denominator), all f32r matmuls.
  - per (head, t-chunk): S.T = k·q scores transposed [s',t] -> exp (ACT) ->
    z' = v'.T @ E accumulated in PSUM ([65, t]; row 64 = denominator),
    f[t] = gate/denom broadcast to [128,t], A.T tile = E*f -> DMA out,
    z = z'[0:64]*f -> DRAM scratch.
  - out = z @ Wo_hg.T partial, summed on host (row-parallel Wo).
A is returned as a zero-copy transposed view of the device's [s',t] layout.
"""

import os
import numpy as np

B, T, D, H = 4, 2048, 1024, 16
Dh = D // H            # 64
NCORES = 8
HPC = H // 2           # heads per core = 8
DHG = D // 2           # head-group width = 512
TC = 1024              # t-chunk
NTC = T // TC          # 2 chunks
NSJ = T // 128         # 16 s'-tiles
NJT = DHG // 128       # 4 j-tiles

_CACHE = {}
LAST_RESULTS = None


def _maybe_install_profhook():
    """Best-effort NTFF profile hook (image lacks antenv.axon_hooks)."""
    import sys
    import types
    try:
        import antenv
        if "antenv.axon_hooks" in sys.modules:
            return
        from trn_agent_boot.trn_boot import _ntff_profile_via_ctypes
        hook = _ntff_profile_via_ctypes("/opt/axon/libaxon_pjrt.so")
        mod = types.ModuleType("antenv.axon_hooks")
        mod.get_axon_ntff_profile_hook = lambda: hook
        mod.set_axon_ntff_profile_hook = lambda h: None
        sys.modules["antenv.axon_hooks"] = mod
        antenv.axon_hooks = mod
    except Exception:
        pass


def _build():
    from contextlib import ExitStack
    from concourse import bacc
    import concourse.tile as tile
    from concourse import mybir

    f32 = mybir.dt.float32
    f32r = mybir.dt.float32r
    Exp = mybir.ActivationFunctionType.Exp
    Ident = mybir.ActivationFunctionType.Identity
    MUL = mybir.AluOpType.mult

    nc = bacc.Bacc("TRN2", target_bir_lowering=False, debug=False)

    xT = nc.dram_tensor("xT", [D, T], f32r, kind="ExternalInput").ap()
    wqT = nc.dram_tensor("wqT", [D, DHG], f32r, kind="ExternalInput").ap()
    wkT = nc.dram_tensor("wkT", [D, DHG], f32r, kind="ExternalInput").ap()
    wvT = nc.dram_tensor("wvT", [D, DHG], f32r, kind="ExternalInput").ap()
    woT = nc.dram_tensor("woT", [DHG, D], f32r, kind="ExternalInput").ap()
    bq_t = nc.dram_tensor("bq_t", [128, NJT], f32, kind="ExternalInput").ap()
    bk_t = nc.dram_tensor("bk_t", [128, NJT], f32, kind="ExternalInput").ap()
    gates_t = nc.dram_tensor("gates_t", [1, HPC], f32, kind="ExternalInput").ap()

    at_out = nc.dram_tensor("at_out", [HPC, T, T], f32, kind="ExternalOutput").ap()
    o_out = nc.dram_tensor("o_out", [T, D], f32, kind="ExternalOutput").ap()
    z_scr = nc.dram_tensor("z_scr", [DHG, T], f32r).ap()

    with tile.TileContext(nc) as tc, ExitStack() as ctx:
        # ---- persistent pools (projections output, consumed by attention) ----
        per = ctx.enter_context(tc.tile_pool(name="per", bufs=1))
        qt_p = ctx.enter_context(tc.tile_pool(name="qt", bufs=NJT))
        kt_p = ctx.enter_context(tc.tile_pool(name="kt", bufs=NJT))
        vp_p = ctx.enter_context(tc.tile_pool(name="vp", bufs=NSJ))

        gt = per.tile([1, HPC], f32, tag="gt")
        nc.sync.dma_start(gt[:], gates_t[:])
        bqt = per.tile([128, NJT], f32, tag="bqt")
        nc.sync.dma_start(bqt[:], bq_t[:])
        bkt = per.tile([128, NJT], f32, tag="bkt")
        nc.sync.dma_start(bkt[:], bk_t[:])

        qt = [qt_p.tile([128, T], f32r, tag="qt", name=f"qt{i}") for i in range(NJT)]
        kt = [kt_p.tile([128, T], f32r, tag="kt", name=f"kt{i}") for i in range(NJT)]
        vp = [vp_p.tile([128, HPC * 65], f32r, tag="vp", name=f"vp{i}") for i in range(NSJ)]

        # ---- phase 1: projections ----
        with tc.tile_pool(name="xt", bufs=8) as xt_p, \
             tc.tile_pool(name="w", bufs=16) as w_p, \
             tc.tile_pool(name="pp", bufs=4, space="PSUM") as pp:
            xt = []
            for kc in range(8):
                t_ = xt_p.tile([128, T], f32r, tag="xt", name=f"xt{kc}")
                nc.sync.dma_start(t_[:], xT[kc * 128:(kc + 1) * 128, :])
                xt.append(t_)

            def load_w(w_dram):
                ws = []
                for kc in range(8):
                    t_ = w_p.tile([128, DHG], f32r, tag="w", name=f"w{kc}")
                    nc.sync.dma_start(t_[:], w_dram[kc * 128:(kc + 1) * 128, :])
                    ws.append(t_)
                return ws

            # qT[j, t] / kT[j, t] projections (out transposed)
            for W, bias, dst in ((load_w(wqT), bqt, qt), (load_w(wkT), bkt, kt)):
                for jt in range(NJT):
                    for tcc in range(4):
                        ps = pp.tile([128, 512], f32, tag="pp")
                        for kc in range(8):
                            nc.tensor.matmul(
                                ps[:],
                                W[kc][:, jt * 128:(jt + 1) * 128],
                                xt[kc][:, tcc * 512:(tcc + 1) * 512],
                                start=(kc == 0), stop=(kc == 7),
                            )
                        nc.scalar.activation(
                            dst[jt][:, tcc * 512:(tcc + 1) * 512],
                            ps[:], Ident, bias=bias[:, jt:jt + 1], scale=1.0,
                        )
            # v natural [t, j], into per-head 65-stride layout + ones columns
            wv = load_w(wvT)
            for tt in range(NSJ):
                ps = pp.tile([128, 512], f32, tag="pp")
                for kc in range(8):
                    nc.tensor.matmul(
                        ps[:],
                        xt[kc][:, tt * 128:(tt + 1) * 128],
                        wv[kc][:],
                        start=(kc == 0), stop=(kc == 7),
                    )
                vv = vp[tt][:].rearrange("p (h c) -> p h c", c=65)[:, :, 0:64]
                nc.scalar.copy(vv, ps[:].rearrange("p (h c) -> p h c", c=64))
                for h in range(HPC):
                    nc.vector.memset(vp[tt][:, h * 65 + 64: h * 65 + 65].bitcast(f32), 1.0)

        # ---- phase 2: attention ----
        with tc.tile_pool(name="st", bufs=2, space="PSUM") as st_p, \
             tc.tile_pool(name="zt", bufs=2, space="PSUM") as zt_p, \
             tc.tile_pool(name="e", bufs=17) as e_p, \
             tc.tile_pool(name="fb", bufs=2) as fb_p, \
             tc.tile_pool(name="ast", bufs=3) as ast_p, \
             tc.tile_pool(name="zs", bufs=2) as zs_p, \
             tc.tile_pool(name="fr", bufs=2) as fr_p:
            zstage = {}
            for h in range(HPC):
                jt, po = h // 2, (h % 2) * 64
                for c in range(NTC):
                    if h % 2 == 0:
                        zstage[(jt, c)] = zs_p.tile([128, TC], f32r, tag="zs", name=f"zs{jt}_{c}")
                    zt_ps = zt_p.tile([65, TC], f32, tag="zt")
                    e_tiles = []
                    for sj in range(NSJ):
                        st_ps = st_p.tile([128, TC], f32, tag="st")
                        for nn in range(2):
                            nc.tensor.matmul(
                                st_ps[:, nn * 512:(nn + 1) * 512],
                                kt[jt][po:po + 64, sj * 128:(sj + 1) * 128],
                                qt[jt][po:po + 64, c * TC + nn * 512: c * TC + (nn + 1) * 512],
                                start=True, stop=True,
                            )
                        e_t = e_p.tile([128, TC], f32r, tag="e")
                        nc.scalar.activation(e_t[:], st_ps[:], Exp, scale=0.125)
                        for nn in range(2):
                            nc.tensor.matmul(
                                zt_ps[:, nn * 512:(nn + 1) * 512],
                                vp[sj][:, h * 65:(h + 1) * 65],
                                e_t[:, nn * 512:(nn + 1) * 512],
                                start=(sj == 0), stop=(sj == NSJ - 1),
                            )
                        e_tiles.append(e_t)
                    recip = fr_p.tile([1, TC], f32, tag="recip")
                    nc.vector.reciprocal(recip[:], zt_ps[64:65, :])
                    nc.vector.tensor_scalar(recip[:], recip[:], gt[0:1, h:h + 1], None, MUL)
                    fb = fb_p.tile([128, TC], f32, tag="fb")
                    nc.gpsimd.partition_broadcast(fb[:], recip[:])
                    for sj in range(NSJ):
                        a_t = ast_p.tile([128, TC], f32, tag="ast")
                        eng = nc.vector if sj % 8 < 5 else nc.gpsimd
                        eng.tensor_tensor(
                            out=a_t[:], in0=e_tiles[sj][:].bitcast(f32), in1=fb[:], op=MUL
                        )
                        nc.sync.dma_start(
                            at_out[h, sj * 128:(sj + 1) * 128, c * TC:(c + 1) * TC], a_t[:]
                        )
                    nc.vector.tensor_tensor(
                        out=zstage[(jt, c)][po:po + 64, :].bitcast(f32),
                        in0=zt_ps[0:64, :], in1=fb[0:64, :], op=MUL,
                    )
                    if h % 2 == 1:
                        nc.sync.dma_start(
                            z_scr[jt * 128:(jt + 1) * 128, c * TC:(c + 1) * TC],
                            zstage[(jt, c)][:],
                        )

        # ---- phase 3: output projection (partial; host sums the two cores) ----
        with tc.tile_pool(name="zi", bufs=4) as zi_p, \
             tc.tile_pool(name="wo", bufs=4) as wo_p, \
             tc.tile_pool(name="ost", bufs=3) as ost_p, \
             tc.tile_pool(name="op", bufs=4, space="PSUM") as op_p:
            zi, wo = [], []
            for jt in range(NJT):
                t_ = zi_p.tile([128, T], f32r, tag="zi", name=f"zi{jt}")
                nc.sync.dma_start(t_[:], z_scr[jt * 128:(jt + 1) * 128, :])
                zi.append(t_)
                t2 = wo_p.tile([128, D], f32r, tag="wo", name=f"wo{jt}")
                nc.sync.dma_start(t2[:], woT[jt * 128:(jt + 1) * 128, :])
                wo.append(t2)
            for tt in range(NSJ):
                ost = ost_p.tile([128, D], f32, tag="ost")
                for dc in range(2):
                    ps = op_p.tile([128, 512], f32, tag="op")
                    for jt in range(NJT):
                        nc.tensor.matmul(
                            ps[:],
                            zi[jt][:, tt * 128:(tt + 1) * 128],
                            wo[jt][:, dc * 512:(dc + 1) * 512],
                            start=(jt == 0), stop=(jt == NJT - 1),
                        )
                    nc.scalar.copy(ost[:, dc * 512:(dc + 1) * 512], ps[:])
                nc.sync.dma_start(o_out[tt * 128:(tt + 1) * 128, :], ost[:])

    nc.compile()
    return nc


def _get_compiled():
    if "nc" not in _CACHE:
        _CACHE["nc"] = _build()
    return _CACHE["nc"]


def kernel(x, gates, Wq, bq, Wk, bk, Wv, bv, Wo, bo):
    global LAST_RESULTS
    if os.environ.get("BASS_TRACE"):
        _maybe_install_profhook()
    from concourse.bass_utils import run_bass_kernel_spmd

    nc = _get_compiled()
    x = np.asarray(x, np.float32)
    gates_v = np.asarray(gates, np.float32)
    Wq_v, Wk_v, Wv_v, Wo_v = (np.asarray(w, np.float32) for w in (Wq, Wk, Wv, Wo))
    bq_v, bk_v, bv_v, bo_v = (np.asarray(b_, np.float32) for b_ in (bq, bk, bv, bo))

    xTs = [np.ascontiguousarray(x[b].T) for b in range(B)]
    in_maps = []
    for c in range(NCORES):
        b, hg = c // 2, c % 2
        sl = slice(hg * DHG, (hg + 1) * DHG)
        in_maps.append({
            "xT": xTs[b],
            "wqT": np.ascontiguousarray(Wq_v[sl, :].T),
            "wkT": np.ascontiguousarray(Wk_v[sl, :].T),
            "wvT": np.ascontiguousarray(Wv_v[sl, :].T),
            "woT": np.ascontiguousarray(Wo_v[:, sl].T),
            "bq_t": np.ascontiguousarray(bq_v[sl].reshape(NJT, 128).T),
            "bk_t": np.ascontiguousarray(bk_v[sl].reshape(NJT, 128).T),
            "gates_t": np.ascontiguousarray(gates_v[b, hg * HPC:(hg + 1) * HPC].reshape(1, HPC)),
        })

    res = run_bass_kernel_spmd(nc, in_maps, core_ids=list(range(NCORES)))
    LAST_RESULTS = res

    out = np.empty((B, T, D), np.float32)
    AT = np.empty((B, H, T, T), np.float32)
    for b in range(B):
        r0, r1 = res.results[2 * b], res.results[2 * b + 1]
        out[b] = r0["o_out"] + r1["o_out"] + bo_v
        # exact bv correction: z += softmax-row-sum (=1) * gate * bv
        out[b] += Wo_v @ (np.repeat(gates_v[b], Dh) * bv_v)
        AT[b, 0:HPC] = r0["at_out"]
        AT[b, HPC:H] = r1["at_out"]
    return out, AT.transpose(0, 1, 3, 2)
